# revision 1
# baseline (speedup 1.0000x reference)
"""CRF tagger loss (forward-algorithm log-partition minus gold path score)
on 8 Trainium2 NeuronCores.

Strategy
--------
Data-parallel over batch (4 shards of 256) x time-direction (forward /
backward halves of the T=1024 sequence), one core per (shard, direction).

The CRF forward recurrence is computed in *linear* space:
    X_{s+1} = F_s * (W @ X_s)
where W = exp(transitions).T (block-diagonal, 5 batch groups stacked on the
partition axis -> state tile [110, 52]) stays numerically bounded via a
global pre-scale exp(-mu) plus a per-column renormalization every 64 steps
(selector-matmul broadcast + reciprocal).  Backward cores run the same
uniform program on time-reversed features with the transposed operator; the
extra trailing matmul output gives beta.  Host combines alpha/beta in
float64.

Gold path score on device: emission term via fused one-hot * feats
multiply-reduce (scalar_tensor_tensor), transition term via one-hot
count-matrix matmuls accumulated in PSUM.  O(B) boundary terms (START/STOP/
seam/partition-junk) are fixed up on host.
"""

import sys

for _p in ("/opt/trn_rl_repo",):
    if _p not in sys.path:
        sys.path.insert(0, _p)

from contextlib import ExitStack

import ml_dtypes
import numpy as np

import concourse.bacc as bacc
import concourse.bass as bass
import concourse.mybir as mybir
import concourse.tile as tile
from concourse.bass_utils import run_bass_kernel_spmd

BF16 = ml_dtypes.bfloat16

# Problem geometry (hardcoded per the task spec).
B, T, C = 1024, 1024, 22
START, STOP = C - 2, C - 1
NCORES = 8
BQ = B // 4            # batch rows per core
NG = 5                 # stacked groups on the partition axis
GW = 52                # batch columns per group (5*52 = 260 >= 256, 4 pad)
NPART = NG * C         # 110
NSTEP = T // 2         # 512 time steps per core
NCHUNK = 4
CHUNK_STEPS = NSTEP // NCHUNK      # 128
RENORM = 64
NRENORM = NSTEP // RENORM          # 8
RPP = (BQ * NSTEP) // 128          # gold rows per partition = 1024
IDXPAD = RPP + 1                   # 1025 (one zero pad slot)
GOLD_W = 22 * IDXPAD               # 22550 columns
NMM = RPP // 4                     # 256 count matmuls (4-stacked)
FSL = 8                            # f-score slices per chunk
NFACC = NCHUNK * FSL               # 32 accumulator columns

_CACHE = {}


# --------------------------------------------------------------------------
# Device program (identical for all 8 cores; roles differ via input data)
# --------------------------------------------------------------------------

def _gold_chunk_range(k):
    """Gold idx range [start, end) per chunk; count-MM group m needs idx
    [4m, 4m+4], groups m in [64k, 64k+64) -> idx [256k, 256k+261)."""
    start = 256 * k
    end = min(start + 261, IDXPAD)
    return start, end


def _build_program(variant="full"):
    f32 = mybir.dt.float32
    bf16 = mybir.dt.bfloat16
    nc = bacc.Bacc("TRN2", target_bir_lowering=False, debug=False,
                   num_devices=NCORES)

    ins = {
        "ftt": nc.dram_tensor("ftt", [NPART, NSTEP * GW], bf16,
                              kind="ExternalInput"),
        "ohc": nc.dram_tensor("ohc", [128, GOLD_W], bf16,
                              kind="ExternalInput"),
        "fnat": nc.dram_tensor("fnat", [128, GOLD_W], bf16,
                               kind="ExternalInput"),
        "wst": nc.dram_tensor("wst", [NPART, NPART], bf16,
                              kind="ExternalInput"),
        "wsel": nc.dram_tensor("wsel", [NPART, NPART], bf16,
                               kind="ExternalInput"),
        "x0": nc.dram_tensor("x0", [NPART, GW], bf16, kind="ExternalInput"),
    }
    outs = {
        "xfin": nc.dram_tensor("xfin", [NPART, GW], bf16,
                               kind="ExternalOutput"),
        "yfin": nc.dram_tensor("yfin", [NPART, GW], f32,
                               kind="ExternalOutput"),
        "divs": nc.dram_tensor("divs", [NPART, NRENORM * GW], f32,
                               kind="ExternalOutput"),
        "cnts": nc.dram_tensor("cnts", [88, 88], f32, kind="ExternalOutput"),
        "facc": nc.dram_tensor("facc", [128, NFACC], f32,
                               kind="ExternalOutput"),
    }

    with tile.TileContext(nc) as tc:
        with ExitStack() as ctx:
            with nc.allow_low_precision(
                    reason="bf16 state is intentional; bookkeeping via "
                           "exact bf16 reciprocal dumps"):
                _emit_body(ctx, tc, ins, outs, variant)

    nc.compile()
    return nc


def _emit_body(ctx, tc, ins, outs, variant="full"):
    f32 = mybir.dt.float32
    bf16 = mybir.dt.bfloat16
    nc = tc.nc
    mult = mybir.AluOpType.mult
    do_gold = True
    do_renorm = True
    do_exp = True
    nch = {"chains2": 2, "chains3": 3, "chains4": 4}.get(variant, 1)
    # chain column ranges partitioning [0, GW)
    base = GW // nch
    widths = [base + (1 if i < GW % nch else 0) for i in range(nch)]
    edges = [0]
    for w in widths:
        edges.append(edges[-1] + w)
    spans = list(zip(edges[:-1], edges[1:]))

    const_pool = ctx.enter_context(tc.tile_pool(name="const", bufs=1))
    state_pool = ctx.enter_context(tc.tile_pool(name="state", bufs=1))
    ft_pool = ctx.enter_context(tc.tile_pool(name="ft", bufs=2))
    f_pool = ctx.enter_context(tc.tile_pool(name="fexp", bufs=2))
    oh_pool = ctx.enter_context(tc.tile_pool(name="oh", bufs=2))
    fn_pool = ctx.enter_context(tc.tile_pool(name="fn", bufs=2))
    scr_pool = ctx.enter_context(tc.tile_pool(name="scr", bufs=2))
    m_psum = ctx.enter_context(tc.tile_pool(name="mps", bufs=1, space="PSUM"))
    r_psum = ctx.enter_context(tc.tile_pool(name="rps", bufs=2, space="PSUM"))
    c_psum = ctx.enter_context(tc.tile_pool(name="cps", bufs=1, space="PSUM"))

    wst = const_pool.tile([NPART, NPART], bf16)
    nc.sync.dma_start(out=wst[:], in_=ins["wst"].ap())
    wsel = const_pool.tile([NPART, NPART], bf16)
    nc.sync.dma_start(out=wsel[:], in_=ins["wsel"].ap())

    Xs = []
    for ci, (c0, c1) in enumerate(spans):
        Xc = state_pool.tile([NPART, c1 - c0], bf16, tag=f"x{ci}")
        nc.sync.dma_start(out=Xc[:], in_=ins["x0"].ap()[:, c0:c1])
        Xs.append(Xc)
    divd = state_pool.tile([NPART, NRENORM * GW], f32)
    faccs = state_pool.tile([128, NFACC], f32)
    cnt_ps = c_psum.tile([88, 88], f32)

    n_cnt_done = 0
    for k in range(NCHUNK):
        # ---- stream in this chunk's transposed features, exponentiate ----
        ft = ft_pool.tile([NPART, CHUNK_STEPS * GW], bf16, tag="ft")
        nc.sync.dma_start(
            out=ft[:],
            in_=ins["ftt"].ap()[:, k * CHUNK_STEPS * GW:(k + 1) * CHUNK_STEPS * GW],
        )
        F = f_pool.tile([NPART, CHUNK_STEPS * GW], f32, tag="fexp")
        if do_exp:
            nc.scalar.activation(F[:], ft[:], mybir.ActivationFunctionType.Exp)

        # ---- stream in this chunk's gold-score data ----
        g0, g1 = _gold_chunk_range(k)
        gw = g1 - g0
        oh = oh_pool.tile([128, gw * 22], bf16, tag="oh")
        nc.sync.dma_start(out=oh[:], in_=ins["ohc"].ap()[:, g0 * 22:g1 * 22])
        fn = fn_pool.tile([128, gw * 22], bf16, tag="fn")
        nc.sync.dma_start(out=fn[:], in_=ins["fnat"].ap()[:, g0 * 22:g1 * 22])

        for s in range(CHUNK_STEPS):
            gs = k * CHUNK_STEPS + s
            # recurrence step per chain: X <- F_s * (W @ X)
            for ci, (c0, c1) in enumerate(spans):
                Xc = Xs[ci]
                M = m_psum.tile([NPART, c1 - c0], f32, tag=f"mps{ci}")
                nc.tensor.matmul(M[:], wst[:], Xc[:], start=True, stop=True)
                nc.vector.tensor_tensor(
                    Xc[:], M[:], F[:, s * GW + c0:s * GW + c1], mult)

            # interleave one count-matmul every other step
            if do_gold and gs % 2 == 0:
                m = gs // 2
                l = 4 * (m - 64 * k)          # local idx of lhsT block
                nc.tensor.matmul(
                    cnt_ps[:],
                    oh[:, 22 * l:22 * l + 88],
                    oh[:, 22 * (l + 1):22 * (l + 1) + 88],
                    start=(n_cnt_done == 0),
                    stop=(n_cnt_done == NMM - 1),
                    skip_group_check=True,
                )
                n_cnt_done += 1

            # interleave f-score slices (8 per chunk over idx [0, 256))
            if do_gold and s % 16 == 15:
                j = s // 16
                lo = 32 * j
                hi = 32 * (j + 1)
                if k == NCHUNK - 1 and j == FSL - 1:
                    hi = gw            # include the zero pad slot
                sl = slice(22 * lo, 22 * hi)
                stt_out = scr_pool.tile([128, 22 * (hi - lo)], bf16,
                                        tag="scr")
                nc.vector.scalar_tensor_tensor(
                    stt_out[:], oh[:, sl], 1.0, fn[:, sl],
                    mybir.AluOpType.mult, mult,
                    accum_out=faccs[:, k * FSL + j:k * FSL + j + 1],
                )

            # periodic per-column renormalization
            if s % RENORM == RENORM - 1:
                r = gs // RENORM
                for ci, (c0, c1) in enumerate(spans):
                    Xc = Xs[ci]
                    R = r_psum.tile([NPART, c1 - c0], f32, tag="rps")
                    nc.tensor.matmul(R[:], wsel[:], Xc[:], start=True,
                                     stop=True)
                    rec = scr_pool.tile([NPART, c1 - c0], bf16,
                                        tag=f"rec{ci}")
                    nc.vector.reciprocal(rec[:], R[:])
                    # dump the *applied* reciprocal (exact bf16->f32 upcast)
                    # so host bookkeeping is exact despite recip error
                    nc.vector.tensor_copy(
                        divd[:, r * GW + c0:r * GW + c1], rec[:])
                    nc.vector.tensor_tensor(Xc[:], Xc[:], rec[:], mult)

    # trailing matmul (beta for backward cores), evict and write everything
    xsb = state_pool.tile([NPART, GW], bf16, tag="xsb")
    ysb = state_pool.tile([NPART, GW], f32, tag="ysb")
    for ci, (c0, c1) in enumerate(spans):
        Xc = Xs[ci]
        Y = m_psum.tile([NPART, c1 - c0], f32, tag=f"mps{ci}")
        nc.tensor.matmul(Y[:], wst[:], Xc[:], start=True, stop=True)
        nc.vector.tensor_copy(ysb[:, c0:c1], Y[:])
        nc.vector.tensor_copy(xsb[:, c0:c1], Xc[:])
    csb = state_pool.tile([88, 88], f32)
    nc.vector.tensor_copy(csb[:], cnt_ps[:])

    nc.sync.dma_start(out=outs["xfin"].ap(), in_=xsb[:])
    nc.sync.dma_start(out=outs["yfin"].ap(), in_=ysb[:])
    nc.sync.dma_start(out=outs["divs"].ap(), in_=divd[:])
    nc.sync.dma_start(out=outs["cnts"].ap(), in_=csb[:])
    nc.sync.dma_start(out=outs["facc"].ap(), in_=faccs[:])


# --------------------------------------------------------------------------
# Host-side sharding / input prep
# --------------------------------------------------------------------------

def _host_consts(transitions):
    tr = np.asarray(transitions, np.float64)
    E = np.exp(tr)                      # [22, 22]; col START and row STOP = 0
    colsum = E.sum(0)
    ok = np.arange(C) != START
    mu = float(np.mean(np.log(np.maximum(colsum[ok], 1e-300))))
    Ep = (E * np.exp(-mu)).astype(np.float32)
    return Ep, mu


def _block_diag(blk):
    out = np.zeros((NPART, NPART), np.float32)
    for g in range(NG):
        out[22 * g:22 * g + 22, 22 * g:22 * g + 22] = blk
    return out


def _core_inputs(core, feats, tags, Ep, mu):
    """Build the 6 device input arrays for one core."""
    q, is_bwd = core % 4, core >= 4
    b0 = q * BQ
    if is_bwd:
        fsl = feats[b0:b0 + BQ, T - 1:NSTEP - 1:-1, :]   # t = 1023..512
        tsl = tags[b0:b0 + BQ, T - 1:NSTEP - 1:-1]
    else:
        fsl = feats[b0:b0 + BQ, :NSTEP, :]               # t = 0..511
        tsl = tags[b0:b0 + BQ, :NSTEP]
    fsl = np.ascontiguousarray(fsl, np.float32)

    # transposed feature layout [110, 512*52]
    padded = np.zeros((NG * GW, NSTEP, C), np.float32)
    padded[:BQ] = fsl
    ftt = np.ascontiguousarray(
        padded.reshape(NG, GW, NSTEP, C).transpose(0, 3, 2, 1)
    ).reshape(NPART, NSTEP * GW).astype(BF16)

    # gold-score natural layout [128, 1025, 22]
    tgp = np.ascontiguousarray(tsl).reshape(128, RPP)
    ohc = np.zeros((128, IDXPAD, C), BF16)
    p_i, r_i = np.meshgrid(np.arange(128), np.arange(RPP), indexing="ij")
    ohc[p_i, r_i, tgp] = BF16(1.0)
    fnat = np.zeros((128, IDXPAD, C), BF16)
    fnat[:, :RPP, :] = fsl.reshape(128, RPP, C).astype(BF16)

    wst = _block_diag(Ep if not is_bwd else Ep.T).astype(BF16)
    wsel = np.zeros((NPART, NPART), np.float32)
    for g in range(NG):
        wsel[22 * g, 22 * g:22 * g + 22] = 1.0        # lhsT row i=r0 -> all j
    wsel = wsel.astype(BF16)

    x0 = np.zeros((NPART, GW), np.float32)
    for g in range(NG):
        if is_bwd:
            x0[22 * g + STOP, :] = 32.0     # exact in bf16; see _combine
        else:
            x0[22 * g + START, :] = 1.0
    x0 = x0.astype(BF16)

    return {
        "ftt": ftt,
        "ohc": ohc.reshape(128, GOLD_W),
        "fnat": fnat.reshape(128, GOLD_W),
        "wst": wst,
        "wsel": wsel,
        "x0": x0,
    }


# --------------------------------------------------------------------------
# Host-side combine
# --------------------------------------------------------------------------

def _log_state(xfin, divs, const, mu):
    """Reconstruct log-state [22, 256-batch] from a core's outputs.
    divs holds the bf16 reciprocals actually multiplied into the state, so
    the offset is exactly -sum(ln rec) plus the mu/init constant."""
    with np.errstate(divide="ignore"):
        lx = np.log(np.asarray(xfin, np.float64))          # [110, 52]
        ld = np.log(np.asarray(divs, np.float64).reshape(NPART, NRENORM, GW))
    out = np.full((C, BQ), -np.inf)
    for g in range(NG):
        bw = GW if g < NG - 1 else BQ - g * GW
        off = -ld[22 * g, :, :bw].sum(0) + const           # [bw]
        out[:, g * GW:g * GW + bw] = lx[22 * g:22 * g + 22, :bw] + off[None]
    return out


def _gold_from_outputs(results, tags, transitions):
    tr = np.asarray(transitions, np.float64)
    f_score = 0.0
    t_score = 0.0
    for core in range(NCORES):
        q, is_bwd = core % 4, core >= 4
        b0 = q * BQ
        r = results[core]
        f_score += float(np.asarray(r["facc"], np.float64).sum())

        c88 = np.asarray(r["cnts"], np.float64)
        c22 = sum(c88[22 * k:22 * k + 22, 22 * k:22 * k + 22]
                  for k in range(4))
        if is_bwd:
            t_score += float((c22 * tr.T).sum())
            tsl = tags[b0:b0 + BQ, T - 1:NSTEP - 1:-1]
        else:
            t_score += float((c22 * tr).sum())
            tsl = tags[b0:b0 + BQ, :NSTEP]
        tgp = np.ascontiguousarray(tsl).reshape(128, RPP)
        # subtract the junk mid-partition pairs (r=511 -> r=512 straddles b)
        a, b = tgp[:, RPP // 2 - 1], tgp[:, RPP // 2]
        if is_bwd:
            t_score -= float(tr[b, a].sum())
        else:
            t_score -= float(tr[a, b].sum())

    # boundary terms not representable on device
    t_score += float(tr[START, tags[:, 0]].sum())
    t_score += float(tr[tags[:, T - 1], STOP].sum())
    t_score += float(tr[tags[np.arange(B), NSTEP - 1],
                        tags[np.arange(B), NSTEP]].sum())
    return f_score, t_score


def _combine(results, tags, transitions, mu):
    tags = np.asarray(tags)
    fwd_score = 0.0
    for q in range(4):
        rf, rb = results[q], results[4 + q]
        # alpha: 512 prescaled matmuls.  beta: 513 matmuls, init vector 32.
        alpha = _log_state(rf["xfin"], rf["divs"], NSTEP * mu, mu)
        beta = _log_state(rb["yfin"], rb["divs"],
                          (NSTEP + 1) * mu - np.log(32.0), mu)
        s = alpha + beta                                   # [22, 256]
        m = s.max(0)
        z = m + np.log(np.exp(s - m[None]).sum(0))
        fwd_score += float(z.sum())
    f_score, t_score = _gold_from_outputs(results, tags, transitions)
    return fwd_score - (t_score + f_score)


# --------------------------------------------------------------------------
# Entry point
# --------------------------------------------------------------------------

def _numpy_reference(feats, mask, tags, transitions):
    """Defensive fallback for inputs the device program doesn't cover."""
    feats = np.asarray(feats, np.float64)
    tags = np.asarray(tags)
    mask = np.asarray(mask)
    tr = np.asarray(transitions, np.float64)
    b, t, c = feats.shape
    alpha = np.full((b, c), -10000.0)
    alpha[:, START] = 0.0
    for i in range(t):
        s = alpha[:, :, None] + feats[:, i, None, :] + tr[None]
        m = s.max(1)
        new = m + np.log(np.exp(s - m[:, None, :]).sum(1))
        alpha = np.where(mask[:, i, None], new, alpha)
    s = alpha + tr[None, :, STOP]
    m = s.max(1)
    fwd = (m + np.log(np.exp(s - m[:, None]).sum(1))).sum()
    seq_len = mask.astype(np.int64).sum(1)
    pad_start = np.concatenate(
        [np.full((b, 1), START, tags.dtype), tags], axis=1)
    pad_stop = np.concatenate(
        [tags, np.full((b, 1), STOP, tags.dtype)], axis=1)
    pad_stop[np.arange(b), seq_len] = STOP
    trv = tr[pad_start, pad_stop]
    t_sc = np.cumsum(trv, 1)[np.arange(b), seq_len].sum()
    emit = np.take_along_axis(feats, tags[:, :, None], axis=2)[:, :, 0]
    f_sc = np.where(mask, emit, 0.0).sum()
    return np.float32(fwd - (t_sc + f_sc))


def _get_program():
    if "nc" not in _CACHE:
        _CACHE["nc"] = _build_program()
    return _CACHE["nc"]


def run_cores(feats, tags, transitions, **spmd_kwargs):
    """Shard, run the 8-core program, return (BassKernelResults, mu)."""
    feats = np.ascontiguousarray(np.asarray(feats, np.float32))
    tags = np.asarray(tags)
    Ep, mu = _host_consts(transitions)
    in_maps = [_core_inputs(core, feats, tags, Ep, mu)
               for core in range(NCORES)]
    nc = _get_program()
    res = run_bass_kernel_spmd(nc, in_maps, core_ids=list(range(NCORES)),
                               **spmd_kwargs)
    return res, mu


def kernel(feats, mask, tags, transitions):
    mask = np.asarray(mask)
    feats = np.asarray(feats)
    tags = np.asarray(tags)
    if feats.shape != (B, T, C) or not mask.all():
        return _numpy_reference(feats, mask, tags, transitions)
    res, mu = run_cores(feats, tags, transitions)
    loss = _combine(res.results, tags, transitions, mu)
    return np.float32(loss)



# revision 7
# speedup vs baseline: 3.6074x; 3.6074x over previous
"""CRF tagger loss (forward-algorithm log-partition minus gold path score)
on 8 Trainium2 NeuronCores.

Strategy
--------
Data-parallel over batch (8 shards of 128 rows) and *time-parallel within
each core*: the T=1024 sequence is split into K=28 chains of L=36 main
steps, each preceded by a 16-step burn-in from a uniform positive vector.
The CRF forward recurrence is strongly contracting in projective metric
(~0.45/step on these inputs), so after 16 burn-in steps the chain state
direction matches the true forward state to ~3e-6; chains then run
concurrently, hiding the ~500 ns cross-engine latency of each serial
recurrence step behind 27 other chains.

The recurrence is computed in linear space,
    X_{s+1} = F_s * (W^T @ X_s),     W = exp(transitions - mu)
with a block-diagonal W (5 batch groups stacked on the partition axis ->
state tile [110, 28*26]) and a single per-column renormalization (by the
group's class-0 row) at the burn-in boundary, which simultaneously
provides the boundary normalization the host-side stitching needs.
Matmuls run per chain on PE; the elementwise multiplies are grouped 6-8
chains per instruction and spread over the Vector and GpSimd engines.

Host side: gold path score computed exactly in float64 (cheap gathers),
per-chain growths stitched into the log-partition in float64.
"""

import sys

for _p in ("/opt/trn_rl_repo",):
    if _p not in sys.path:
        sys.path.insert(0, _p)

from contextlib import ExitStack

import ml_dtypes
import numpy as np

import concourse.bacc as bacc
import concourse.bass as bass
import concourse.mybir as mybir
import concourse.tile as tile
from concourse.bass_utils import run_bass_kernel_spmd

BF16 = ml_dtypes.bfloat16

# Problem geometry (hardcoded per the task spec).
B, T, C = 1024, 1024, 22
START, STOP = C - 2, C - 1
NEG = -10000.0
NCORES = 8
BQ = B // NCORES       # batch rows per core (128)
NG = 5                 # stacked groups on the partition axis
GW = 26                # batch columns per group (5*26 = 130 >= 128)
NPART = NG * C         # 110
BURN = 16              # burn-in steps per chain
K = 28                 # chains per core
L = 36                 # main steps per chain; K*L + BURN == T
S = BURN + L           # total steps per chain (52)
ROWW = K * GW          # row width in columns (728)
CH = 4                 # rows per DMA chunk
ESL = 2                # rows per exp slice
# chain groups: (k0, k1, engine) — multiply instruction per group.
# GPSIMD cannot touch PSUM, so the PSUM-reading multiplies all go on DVE;
# two groups of 14 balance instruction overhead against round-trip latency.
GROUPS = ((0, 14, "v"), (14, 28, "v"))

assert K * L + BURN == T and S % CH == 0 and CH % ESL == 0

_CACHE = {}


# --------------------------------------------------------------------------
# Device program (identical for all 8 cores)
# --------------------------------------------------------------------------

def _build_program():
    f32 = mybir.dt.float32
    bf16 = mybir.dt.bfloat16
    nc = bacc.Bacc("TRN2", target_bir_lowering=False, debug=False,
                   num_devices=NCORES)

    ins = {
        "ftt": nc.dram_tensor("ftt", [NPART, S * ROWW], bf16,
                              kind="ExternalInput"),
        "wst": nc.dram_tensor("wst", [NPART, NPART], bf16,
                              kind="ExternalInput"),
        "wsel": nc.dram_tensor("wsel", [NPART, NPART], bf16,
                               kind="ExternalInput"),
        "x0": nc.dram_tensor("x0", [NPART, ROWW], bf16,
                             kind="ExternalInput"),
    }
    outs = {
        "xfin": nc.dram_tensor("xfin", [NPART, ROWW], f32,
                               kind="ExternalOutput"),
        "divs": nc.dram_tensor("divs", [NPART, ROWW], f32,
                               kind="ExternalOutput"),
    }

    with tile.TileContext(nc) as tc:
        with ExitStack() as ctx:
            with nc.allow_low_precision(
                    reason="bf16 state is intentional; bookkeeping via "
                           "exact f32 reciprocal dumps"):
                _emit_body(ctx, tc, ins, outs)

    nc.compile()
    return nc


def _emit_body(ctx, tc, ins, outs):
    f32 = mybir.dt.float32
    bf16 = mybir.dt.bfloat16
    nc = tc.nc
    mult = mybir.AluOpType.mult

    const_pool = ctx.enter_context(tc.tile_pool(name="const", bufs=1))
    state_pool = ctx.enter_context(tc.tile_pool(name="state", bufs=1))
    ft_pool = ctx.enter_context(tc.tile_pool(name="ft", bufs=3))
    f_pool = ctx.enter_context(tc.tile_pool(name="fexp", bufs=3))
    mm_psum = ctx.enter_context(tc.tile_pool(name="mps", bufs=2,
                                             space="PSUM"))

    wst = const_pool.tile([NPART, NPART], bf16)
    nc.sync.dma_start(out=wst[:], in_=ins["wst"].ap())
    wsel = const_pool.tile([NPART, NPART], bf16)
    nc.sync.dma_start(out=wsel[:], in_=ins["wsel"].ap())
    X = state_pool.tile([NPART, ROWW], bf16)
    nc.sync.dma_start(out=X[:], in_=ins["x0"].ap())
    divd = state_pool.tile([NPART, ROWW], f32)

    F = None
    for s in range(S):
        c, r = divmod(s, CH)
        if r == 0:
            ft = ft_pool.tile([NPART, CH * ROWW], bf16, tag="ft")
            nc.sync.dma_start(
                out=ft[:],
                in_=ins["ftt"].ap()[:, c * CH * ROWW:(c + 1) * CH * ROWW])
            F = f_pool.tile([NPART, CH * ROWW], bf16, tag="fexp")
            for e in range(CH // ESL):
                sl = slice(e * ESL * ROWW, (e + 1) * ESL * ROWW)
                nc.scalar.activation(F[:, sl], ft[:, sl],
                                     mybir.ActivationFunctionType.Exp)

        for (k0, k1, eng) in GROUPS:
            gw = (k1 - k0) * GW
            Mg = mm_psum.tile([NPART, gw], f32, tag=f"mm{k0}")
            # all chains share wst -> one merged matmul per group
            nc.tensor.matmul(Mg[:], wst[:], X[:, k0 * GW:k1 * GW],
                             start=True, stop=True)
            engine = nc.vector if eng == "v" else nc.gpsimd
            engine.tensor_tensor(
                X[:, k0 * GW:k1 * GW], Mg[:],
                F[:, r * ROWW + k0 * GW:r * ROWW + k1 * GW], mult)

        if s == BURN - 1:
            # boundary renorm: divide every chain column by its group's
            # class-0 row value; record the applied reciprocal in divd
            for (k0, k1, eng) in GROUPS:
                gw = (k1 - k0) * GW
                Rg = mm_psum.tile([NPART, gw], f32, tag=f"mm{k0}")
                nc.tensor.matmul(Rg[:], wsel[:], X[:, k0 * GW:k1 * GW],
                                 start=True, stop=True)
                nc.vector.reciprocal(divd[:, k0 * GW:k1 * GW], Rg[:])
            for (k0, k1, eng) in GROUPS:
                sl = slice(k0 * GW, k1 * GW)
                nc.vector.tensor_tensor(X[:, sl], X[:, sl], divd[:, sl],
                                        mult)

    xsb = state_pool.tile([NPART, ROWW], f32)
    nc.vector.tensor_copy(xsb[:], X[:])
    nc.sync.dma_start(out=outs["xfin"].ap(), in_=xsb[:])
    nc.sync.dma_start(out=outs["divs"].ap(), in_=divd[:])


# --------------------------------------------------------------------------
# Host-side sharding / input prep
# --------------------------------------------------------------------------

def _host_consts(transitions):
    tr = np.asarray(transitions, np.float64)
    E = np.exp(tr)                      # [22, 22]; col START and row STOP = 0
    colsum = E.sum(0)
    ok = np.arange(C) != START
    mu = float(np.mean(np.log(np.maximum(colsum[ok], 1e-300))))
    Ep = (E * np.exp(-mu)).astype(np.float32)
    return Ep, mu


def _block_diag(blk):
    out = np.zeros((NPART, NPART), np.float32)
    for g in range(NG):
        out[C * g:C * g + C, C * g:C * g + C] = blk
    return out


def _shared_consts(Ep):
    wst = _block_diag(Ep).astype(BF16)
    wsel = np.zeros((NPART, NPART), np.float32)
    for g in range(NG):
        wsel[C * g, C * g:C * g + C] = 1.0
    wsel = wsel.astype(BF16)

    x0 = np.ones((NPART, K, GW), np.float32)
    for g in range(NG):
        x0[C * g:C * g + C, 0, :] = 0.0
        x0[C * g + START, 0, :] = 1.0      # chain 0 starts exactly at START
    x0 = x0.reshape(NPART, ROWW).astype(BF16)
    return wst, wsel, x0


def _core_inputs(core, feats, wst, wsel, x0):
    """Build the device input arrays for one core."""
    fsl = feats[core * BQ:(core + 1) * BQ]           # [128, T, C] f32
    pad = np.zeros((NG * GW, T, C), np.float32)
    pad[:BQ] = fsl
    pad = pad.reshape(NG, GW, T, C)
    # ftt[22g+c, (s*K + k)*GW + j] = pad[g, j, k*L + s, c]
    sw = np.lib.stride_tricks.sliding_window_view(pad, S, axis=2)
    sw = sw[:, :, ::L, :, :]                         # [NG, GW, K, C, S]
    ftt = np.ascontiguousarray(
        sw.transpose(0, 3, 4, 2, 1), dtype=BF16).reshape(NPART, S * ROWW)
    return {"ftt": ftt, "wst": wst, "wsel": wsel, "x0": x0}


# --------------------------------------------------------------------------
# Host-side combine
# --------------------------------------------------------------------------

def _gold_host(feats, tags, transitions):
    tr = np.asarray(transitions, np.float64)
    tags = np.asarray(tags)
    t_sc = tr[START, tags[:, 0]].sum() + tr[tags[:, -1], STOP].sum()
    t_sc += tr[tags[:, :-1], tags[:, 1:]].sum()
    f_sc = np.take_along_axis(
        np.asarray(feats, np.float64), tags[:, :, None], axis=2).sum()
    return float(t_sc + f_sc)


def _combine(results, feats, tags, transitions, mu):
    tr = np.asarray(transitions, np.float64)
    lu = tr[:, STOP]                                  # log of STOP weights
    fwd = 0.0
    for core in range(NCORES):
        r = results[core]
        with np.errstate(divide="ignore"):
            lx = np.log(np.asarray(r["xfin"], np.float64)).reshape(
                NG, C, K, GW)
            ld = np.log(np.asarray(r["divs"], np.float64)).reshape(
                NG, C, K, GW)[:, 0, :, :]             # [NG, K, GW]
        for g in range(NG):
            ncols = min(GW, BQ - g * GW)
            if ncols <= 0:
                break
            # chains 0..K-2: class-0 growth; chain 0 counts burn-in + renorm
            fwd += float((S * mu - ld[g, 0, :ncols]
                          + lx[g, 0, 0, :ncols]).sum())
            fwd += float((K - 2) * ncols * L * mu
                         + lx[g, 0, 1:K - 1, :ncols].sum())
            # last chain: logsumexp with STOP transition
            v = lx[g, :, K - 1, :ncols] + lu[:, None]
            m = v.max(0)
            lse = m + np.log(np.exp(v - m[None]).sum(0))
            fwd += float((lse + L * mu).sum())
    return fwd - _gold_host(feats, tags, transitions)


# --------------------------------------------------------------------------
# Entry point
# --------------------------------------------------------------------------

def _numpy_reference(feats, mask, tags, transitions):
    """Defensive fallback for inputs the device program doesn't cover."""
    feats = np.asarray(feats, np.float64)
    tags = np.asarray(tags)
    mask = np.asarray(mask)
    tr = np.asarray(transitions, np.float64)
    b, t, c = feats.shape
    alpha = np.full((b, c), NEG)
    alpha[:, START] = 0.0
    for i in range(t):
        s = alpha[:, :, None] + feats[:, i, None, :] + tr[None]
        m = s.max(1)
        new = m + np.log(np.exp(s - m[:, None, :]).sum(1))
        alpha = np.where(mask[:, i, None], new, alpha)
    s = alpha + tr[None, :, STOP]
    m = s.max(1)
    fwd = (m + np.log(np.exp(s - m[:, None]).sum(1))).sum()
    seq_len = mask.astype(np.int64).sum(1)
    pad_start = np.concatenate(
        [np.full((b, 1), START, tags.dtype), tags], axis=1)
    pad_stop = np.concatenate(
        [tags, np.full((b, 1), STOP, tags.dtype)], axis=1)
    pad_stop[np.arange(b), seq_len] = STOP
    trv = tr[pad_start, pad_stop]
    t_sc = np.cumsum(trv, 1)[np.arange(b), seq_len].sum()
    emit = np.take_along_axis(feats, tags[:, :, None], axis=2)[:, :, 0]
    f_sc = np.where(mask, emit, 0.0).sum()
    return np.float32(fwd - (t_sc + f_sc))


def _get_program():
    if "nc" not in _CACHE:
        _CACHE["nc"] = _build_program()
    return _CACHE["nc"]


def run_cores(feats, tags, transitions, **spmd_kwargs):
    """Shard, run the 8-core program, return (BassKernelResults, mu)."""
    feats = np.ascontiguousarray(np.asarray(feats, np.float32))
    Ep, mu = _host_consts(transitions)
    wst, wsel, x0 = _shared_consts(Ep)
    in_maps = [_core_inputs(core, feats, wst, wsel, x0)
               for core in range(NCORES)]
    nc = _get_program()
    res = run_bass_kernel_spmd(nc, in_maps, core_ids=list(range(NCORES)),
                               **spmd_kwargs)
    return res, mu


def kernel(feats, mask, tags, transitions):
    mask = np.asarray(mask)
    feats = np.asarray(feats)
    tags = np.asarray(tags)
    if feats.shape != (B, T, C) or not mask.all():
        return _numpy_reference(feats, mask, tags, transitions)
    res, mu = run_cores(feats, tags, transitions)
    loss = _combine(res.results, feats, tags, transitions, mu)
    return np.float32(loss)
